# revision 37
# baseline (speedup 1.0000x reference)
"""Bahdanau attention kernel for Trainium2 (Bass/Tile), 8 NeuronCores.

Problem (per batch element b):
    q_proj = query[b] @ w1.T          # (LQ, H)
    k_proj = key[b]   @ w2.T          # (LK, H)
    score[q, k] = sum_h v[h] * tanh(q_proj[q, h] + k_proj[k, h])
    attn = softmax(score, axis=-1)    # output 1
    ctx  = attn @ value[b]            # output 2

Sharding: data-parallel over batch B=8 across the 8 cores (no collectives).

Algorithm: instead of materializing the (LQ, LK, H) tanh tensor (33.5M
elements/core, ACT-bound at ~200us), expand tanh in a sine series fitted
under the Gaussian input distribution (weighted nonlinear LSQ offline):

    tanh(x) ~= sum_m beta_m sin(omega_m x)
    sin(w(a+b)) = sin(wa)cos(wb) + cos(wa)sin(wb)

so the score becomes a rank-(2*M*H) matmul between per-side feature maps
of the SMALL (H, L) projections.  Weighted-RMS fit error 1.2e-3 (M=6),
below the bf16 feature-quantization floor.

The HW Sin activation has no range reduction (valid only |arg| <~ pi), so
arguments are reduced explicitly per frequency with exact fp32 arithmetic:
    t  = qkp * (omega/2pi)                 (Pool)
    u  = (t + 1.5*2^23) - 1.5*2^23         (Pool; IEEE round-to-nearest)
    rc = t - u  in [-0.5, 0.5]             (DVE)
    sin = Sin(2pi * rc)                    (ACT, bf16)
    h   = Sin(pi * rc)                     (ACT, fp16)
    h2  = h * h                            (DVE, fp16)
    cos_scaled = vbeta - 2*vbeta*h2        (DVE; = v*beta*cos(w x), bf16)
The v_h*beta_m weights ride on both cos maps (one per side), so sin maps
stay unscaled and each m needs only 2 ACT + 3 DVE/Pool-pairable ALU ops.

Softmax uses a constant bias (scores are bounded, |score| < 4, and any
constant cancels in softmax), so there is no row-max reduction; exp writes
bf16 p directly (bf16 transposes, no casts) with an fp32 accumulator for
the row sums.  Outputs are written bf16 and upcast on the host.
"""

import numpy as np

import concourse.bass as bass
import concourse.mybir as mybir
import concourse.tile as tile
from concourse import bacc
from concourse.bass_utils import run_bass_kernel_spmd
from concourse.masks import make_identity

F32 = mybir.dt.float32
BF16 = mybir.dt.bfloat16
FP16 = mybir.dt.float16

B = 8
L = 512          # LQ == LK
D = 512          # DQ == DK == DV
H = 128
P = 128          # SBUF partitions
NDB = D // P     # 4 d-blocks
NQB = L // P     # 4 query blocks

# Optimized sine fit of tanh (offline, Gaussian weight sigma=1.665 + floor,
# range +-11; see module docstring).  The frequencies are stored as
# omega/2pi values EXACT in bf16, so the PE diag matmul that scales the
# projections introduces no frequency error; beta is refit against them.
M_FREQ = 5
WP = np.array([0.041259765625, 0.1259765625, 0.2265625,
               0.353515625, 0.5078125])           # omega_m / 2pi, bf16-exact
BETA = np.array([1.2385136240851755, 0.35480276158259183,
                 0.15381870648783913, 0.05379897839170714,
                 0.01455732383174043])

TWO_PI = float(2 * np.pi)
RND_C = float(1.5 * 2 ** 23)   # fp32 magic rounding constant
EXP_BIAS = -4.0                # constant softmax shift (cancels in normalize)

_CACHED_NC = None


def _build_nc():
    nc = bacc.Bacc("TRN2", target_bir_lowering=False, debug=False)

    # All inputs arrive pre-tiled on the host so each SBUF partition's data is
    # one contiguous DRAM line.
    qT = nc.dram_tensor("qT", [P, NDB, L], BF16, kind="ExternalInput")
    kT = nc.dram_tensor("kT", [P, NDB, L], BF16, kind="ExternalInput")
    val = nc.dram_tensor("val", [P, NQB, D], BF16, kind="ExternalInput")
    w1T = nc.dram_tensor("w1T", [P, NDB, H], BF16, kind="ExternalInput")
    w2T = nc.dram_tensor("w2T", [P, NDB, H], BF16, kind="ExternalInput")
    # vbeta[h, m] = v[h]*beta[m]; vbeta2 = -2*vbeta
    vbeta = nc.dram_tensor("vbeta", [H, M_FREQ], F32, kind="ExternalInput")
    vbeta2 = nc.dram_tensor("vbeta2", [H, M_FREQ], F32, kind="ExternalInput")
    # dgw[:, m, :] = diag(WP[m]) for the PE argument-scaling matmuls
    dgw = nc.dram_tensor("dgw", [P, M_FREQ, P], BF16, kind="ExternalInput")
    attn = nc.dram_tensor("attn", [L, L], BF16, kind="ExternalOutput")
    ctxo = nc.dram_tensor("ctx", [L, L], BF16, kind="ExternalOutput")

    with tile.TileContext(nc) as tc:
        with (
            tc.tile_pool(name="const", bufs=1) as const,
            tc.tile_pool(name="tu", bufs=2) as tu_pool,
            tc.tile_pool(name="feat", bufs=3) as feat_pool,
            tc.tile_pool(name="p", bufs=4) as p_pool,
            tc.tile_pool(name="pt", bufs=8) as pt_pool,
            tc.tile_pool(name="outs", bufs=4) as out_pool,
            tc.tile_pool(name="stat", bufs=8) as stat_pool,
            tc.tile_pool(name="score_ps", bufs=4, space="PSUM") as score_ps_pool,
            tc.tile_pool(name="rc_ps", bufs=2, space="PSUM") as rc_ps_pool,
        ):
            # ---------------- prologue ----------------
            ident = const.tile([P, P], BF16)
            make_identity(nc, ident[:])
            neg4 = const.tile([P, 1], F32)
            nc.vector.memset(neg4[:], EXP_BIAS)
            rndc = const.tile([P, 1], F32)
            nc.vector.memset(rndc[:], RND_C)

            # PE pre-warm on the locally-built identity: starts the busy
            # streak immediately, without waiting for any input DMA, and
            # without delaying the projections behind a long warm queue.
            warm_ps = rc_ps_pool.tile([H, 2 * L], F32, tag="rc", name="warm_ps")
            for _ in range(8):
                nc.tensor.matmul(warm_ps[:, 0:P], ident[:], ident[:])

            w1T_sb = const.tile([P, NDB, H], BF16)
            w2T_sb = const.tile([P, NDB, H], BF16)
            vbeta_sb = const.tile([H, M_FREQ], F32)
            vbeta2_sb = const.tile([H, M_FREQ], F32)
            dgw_sb = const.tile([P, M_FREQ, P], BF16)
            nc.sync.dma_start(out=w1T_sb[:], in_=w1T[:])
            nc.scalar.dma_start(out=w2T_sb[:], in_=w2T[:])

            qT_sb = const.tile([P, NDB, L], BF16)
            kT_sb = const.tile([P, NDB, L], BF16)
            for db in range(NDB):
                nc.sync.dma_start(out=qT_sb[:, db, :], in_=qT[:, db, :])
                nc.scalar.dma_start(out=kT_sb[:, db, :], in_=kT[:, db, :])

            # Small tensors not needed until the m-pipeline: after the
            # projection-critical loads.
            nc.scalar.dma_start(out=vbeta_sb[:], in_=vbeta[:, :])
            nc.scalar.dma_start(out=vbeta2_sb[:], in_=vbeta2[:, :])
            nc.scalar.dma_start(out=dgw_sb[:], in_=dgw[:])

            # value is needed only by the tail context matmuls; load it after
            # the projection inputs, split across both HWDGE queues.
            val_sb = const.tile([P, NQB, D], BF16)
            nc.sync.dma_start(out=val_sb[:, : NQB // 2, :], in_=val[:, : NQB // 2, :])
            nc.scalar.dma_start(out=val_sb[:, NQB // 2 :, :], in_=val[:, NQB // 2 :, :])

            # ---------------- projections: qkp = [qpT | kpT] ----------------
            qkp = const.tile([H, 2 * L], F32)
            ps_q = score_ps_pool.tile([H, L], F32, tag="score", name="ps_q")
            ps_k = score_ps_pool.tile([H, L], F32, tag="score", name="ps_k")
            for db in range(NDB):
                nc.tensor.matmul(
                    ps_q[:], w1T_sb[:, db, :], qT_sb[:, db, :],
                    start=(db == 0), stop=(db == NDB - 1),
                )
                nc.tensor.matmul(
                    ps_k[:], w2T_sb[:, db, :], kT_sb[:, db, :],
                    start=(db == 0), stop=(db == NDB - 1),
                )
            # PSUM->SBUF on ACT (idle here): the first Sin then follows its
            # own producer on the same engine, with no cross-engine hop.
            nc.scalar.copy(qkp[:, 0:L], ps_q[:])
            nc.scalar.copy(qkp[:, L : 2 * L], ps_k[:])
            # hi/lo bf16 split of qkp: diag(wp)*(hi+lo) on the PE reproduces
            # wp*qkp to ~2^-17 relative, at bf16 matmul speed.
            qkp_hi = const.tile([H, 2 * L], BF16)
            qkp_lo = const.tile([H, 2 * L], BF16)
            nc.vector.tensor_copy(qkp_hi[:], qkp[:])
            nc.vector.tensor_tensor(
                qkp_lo[:], qkp[:], qkp_hi[:], mybir.AluOpType.subtract
            )

            # ---------------- m-pipeline ----------------
            score_ps = [
                score_ps_pool.tile([P, L], F32, name=f"score_ps{qb}", tag="score")
                for qb in range(NQB)
            ]

            def emit_scores(m, sin_t, cv_t):
                # score += sin_q x (vb cos_k)  +  (vb cos_q) x sin_k
                for qb in range(NQB):
                    nc.tensor.matmul(
                        score_ps[qb][:],
                        sin_t[:, qb * P : (qb + 1) * P],
                        cv_t[:, L : 2 * L],
                        start=(m == 0), stop=False,
                    )
                    nc.tensor.matmul(
                        score_ps[qb][:],
                        cv_t[:, qb * P : (qb + 1) * P],
                        sin_t[:, L : 2 * L],
                        start=False, stop=(m == M_FREQ - 1),
                    )

            prev = None
            for m in range(M_FREQ):
                wp = float(WP[m])
                if m == 0:
                    # |omega_0 * x| < pi: no range reduction needed.
                    rc_src = qkp[:]
                    s_sin, s_h = TWO_PI * wp, float(np.pi) * wp
                else:
                    # k = round(wp*qkp) via fp32 magic rounding on DVE; the
                    # PE then accumulates rc = wp*(hi+lo) - k in PSUM.
                    ub_t = tu_pool.tile([H, 2 * L], F32, name="ub_t", tag="ub")
                    kneg_t = tu_pool.tile([H, 2 * L], BF16, name="kneg_t",
                                          tag="kneg")
                    # ub = wp*qkp + C (the add rounds): q-half on DVE, k-half
                    # on ACT Identity to balance the two engines.
                    nc.vector.tensor_scalar(
                        ub_t[:, 0:L], qkp[:, 0:L], wp, RND_C,
                        mybir.AluOpType.mult, mybir.AluOpType.add,
                    )
                    nc.scalar.activation(
                        ub_t[:, L : 2 * L], qkp[:, L : 2 * L],
                        mybir.ActivationFunctionType.Identity,
                        bias=rndc[:], scale=wp,
                    )
                    nc.vector.tensor_scalar(
                        kneg_t[:], ub_t[:], -1.0, RND_C,
                        mybir.AluOpType.mult, mybir.AluOpType.add,
                    )
                    rc_ps = rc_ps_pool.tile([H, 2 * L], F32, name="rc_ps",
                                            tag="rc")
                    for half in range(2):
                        sl = slice(half * L, (half + 1) * L)
                        nc.tensor.matmul(
                            rc_ps[:, sl], dgw_sb[:, m, :], qkp_hi[:, sl],
                            start=True, stop=False,
                        )
                        nc.tensor.matmul(
                            rc_ps[:, sl], dgw_sb[:, m, :], qkp_lo[:, sl],
                            start=False, stop=False,
                        )
                        nc.tensor.matmul(
                            rc_ps[:, sl], ident[:], kneg_t[:, sl],
                            start=False, stop=True,
                        )
                    rc_src = rc_ps[:]
                    s_sin, s_h = TWO_PI, float(np.pi)
                sin_t = feat_pool.tile([H, 2 * L], BF16, name="sin_t", tag="sin")
                h_t = feat_pool.tile([H, 2 * L], FP16, name="h_t", tag="h")
                nc.scalar.activation(
                    sin_t[:], rc_src, mybir.ActivationFunctionType.Sin,
                    scale=s_sin,
                )
                nc.scalar.activation(
                    h_t[:], rc_src, mybir.ActivationFunctionType.Sin,
                    scale=s_h,
                )
                h2_t = feat_pool.tile([H, 2 * L], FP16, name="h2_t", tag="h2")
                nc.vector.tensor_tensor(
                    h2_t[:], h_t[:], h_t[:], mybir.AluOpType.mult
                )
                cv_t = feat_pool.tile([H, 2 * L], BF16, name="cv_t", tag="cv")
                nc.vector.tensor_scalar(
                    cv_t[:], h2_t[:],
                    vbeta2_sb[:, m : m + 1], vbeta_sb[:, m : m + 1],
                    mybir.AluOpType.mult, mybir.AluOpType.add,
                )
                if prev is not None:
                    emit_scores(*prev)
                prev = (m, sin_t, cv_t)
            emit_scores(*prev)

            # ---------------- softmax + context per query block -------------
            for qb in range(NQB):
                p_t = p_pool.tile([P, L], BF16)
                sums = stat_pool.tile([P, 1], F32)
                nc.scalar.activation(
                    p_t[:],
                    score_ps[qb][:],
                    mybir.ActivationFunctionType.Exp,
                    bias=neg4[:],
                    accum_out=sums[:],
                )
                inv = stat_pool.tile([P, 1], F32)
                nc.vector.reciprocal(inv[:], sums[:])

                attn_t = out_pool.tile([P, L], BF16)
                nc.vector.tensor_scalar_mul(attn_t[:], p_t[:], inv[:])
                nc.sync.dma_start(
                    out=attn[qb * P : (qb + 1) * P, :], in_=attn_t[:]
                )

                # context: ctx[qb] = (p @ value) * inv
                pT_sbs = []
                for kb in range(NQB):
                    # Transposes rotate through the score banks, which free
                    # up as each block's exp consumes them.
                    tp = score_ps_pool.tile([P, P], BF16, name="tp", tag="score")
                    nc.tensor.transpose(
                        tp[:], p_t[:, kb * P : (kb + 1) * P], ident[:]
                    )
                    pT_sb = pt_pool.tile([P, P], BF16, name="pT_sb", tag="pt")
                    nc.vector.tensor_copy(pT_sb[:], tp[:])
                    pT_sbs.append(pT_sb)
                # ctx accumulators rotate through the (now dead) rc banks.
                ctx_ps = rc_ps_pool.tile([P, D], F32, tag="rc", name="ctx_ps")
                for kb in range(NQB):
                    nc.tensor.matmul(
                        ctx_ps[:],
                        pT_sbs[kb][:],
                        val_sb[:, kb, :],
                        start=(kb == 0),
                        stop=(kb == NQB - 1),
                    )
                ctx_t = out_pool.tile([P, D], BF16)
                nc.vector.tensor_scalar_mul(ctx_t[:], ctx_ps[:], inv[:])
                nc.scalar.dma_start(
                    out=ctxo[qb * P : (qb + 1) * P, :], in_=ctx_t[:]
                )

    nc.compile()
    return nc


def _get_nc():
    global _CACHED_NC
    if _CACHED_NC is None:
        _CACHED_NC = _build_nc()
    return _CACHED_NC


def _in_maps(query, key, value, w1, w2, v):
    import ml_dtypes as _md

    f = np.float32
    bf = _md.bfloat16

    def tile_rows(arr):
        # [R, C] with R = NB*P  ->  [P, NB, C]: partition-major, so each
        # SBUF partition's data is one contiguous DRAM line.
        r, c = arr.shape
        nb = r // P
        return np.ascontiguousarray(arr.reshape(nb, P, c).transpose(1, 0, 2))

    w1T = tile_rows(np.asarray(w1, dtype=f).T.astype(bf))
    w2T = tile_rows(np.asarray(w2, dtype=f).T.astype(bf))
    vb = (np.asarray(v, dtype=np.float64)[0][:, None] * BETA[None, :]).astype(f)
    vb2 = (-2.0 * vb).astype(f)
    dgw = np.zeros((P, M_FREQ, P), dtype=bf)
    for m in range(M_FREQ):
        np.fill_diagonal(dgw[:, m, :], bf(WP[m]))
    maps = []
    for b in range(B):
        maps.append(
            {
                "qT": tile_rows(np.asarray(query[b], dtype=f).T.astype(bf)),
                "kT": tile_rows(np.asarray(key[b], dtype=f).T.astype(bf)),
                "val": tile_rows(np.asarray(value[b], dtype=f).astype(bf)),
                "w1T": w1T,
                "w2T": w2T,
                "vbeta": vb,
                "vbeta2": vb2,
                "dgw": dgw,
            }
        )
    return maps


def run(query, key, value, w1, w2, v, trace=False, **spmd_kwargs):
    nc = _get_nc()
    res = run_bass_kernel_spmd(
        nc,
        _in_maps(query, key, value, w1, w2, v),
        list(range(B)),
        trace=trace,
        **spmd_kwargs,
    )
    attn = np.stack(
        [res.results[b]["attn"].astype(np.float32) for b in range(B)]
    )
    ctx = np.stack(
        [res.results[b]["ctx"].astype(np.float32) for b in range(B)]
    )
    return (attn, ctx), res


def kernel(query, key, value, w1, w2, v):
    (attn, ctx), _ = run(query, key, value, w1, w2, v, trace=False)
    return (attn, ctx)


# revision 39
# speedup vs baseline: 1.0329x; 1.0329x over previous
"""Bahdanau attention kernel for Trainium2 (Bass/Tile), 8 NeuronCores.

Problem (per batch element b):
    q_proj = query[b] @ w1.T          # (LQ, H)
    k_proj = key[b]   @ w2.T          # (LK, H)
    score[q, k] = sum_h v[h] * tanh(q_proj[q, h] + k_proj[k, h])
    attn = softmax(score, axis=-1)    # output 1
    ctx  = attn @ value[b]            # output 2

Sharding: data-parallel over batch B=8 across the 8 cores (no collectives).

Algorithm: instead of materializing the (LQ, LK, H) tanh tensor (33.5M
elements/core, ACT-bound at ~200us), expand tanh in a sine series fitted
under the Gaussian input distribution (weighted nonlinear LSQ offline):

    tanh(x) ~= sum_m beta_m sin(omega_m x)
    sin(w(a+b)) = sin(wa)cos(wb) + cos(wa)sin(wb)

so the score becomes a rank-(2*M*H) matmul between per-side feature maps
of the SMALL (H, L) projections.  Weighted-RMS fit error 1.2e-3 (M=6),
below the bf16 feature-quantization floor.

The HW Sin activation has no range reduction (valid only |arg| <~ pi), so
arguments are reduced explicitly per frequency with exact fp32 arithmetic:
    t  = qkp * (omega/2pi)                 (Pool)
    u  = (t + 1.5*2^23) - 1.5*2^23         (Pool; IEEE round-to-nearest)
    rc = t - u  in [-0.5, 0.5]             (DVE)
    sin = Sin(2pi * rc)                    (ACT, bf16)
    h   = Sin(pi * rc)                     (ACT, fp16)
    h2  = h * h                            (DVE, fp16)
    cos_scaled = vbeta - 2*vbeta*h2        (DVE; = v*beta*cos(w x), bf16)
The v_h*beta_m weights ride on both cos maps (one per side), so sin maps
stay unscaled and each m needs only 2 ACT + 3 DVE/Pool-pairable ALU ops.

Softmax uses a constant bias (scores are bounded, |score| < 4, and any
constant cancels in softmax), so there is no row-max reduction; exp writes
bf16 p directly (bf16 transposes, no casts) with an fp32 accumulator for
the row sums.  Outputs are written bf16 and upcast on the host.
"""

import numpy as np

import concourse.bass as bass
import concourse.mybir as mybir
import concourse.tile as tile
from concourse import bacc
from concourse.bass_utils import run_bass_kernel_spmd
from concourse.masks import make_identity

F32 = mybir.dt.float32
BF16 = mybir.dt.bfloat16
FP16 = mybir.dt.float16

B = 8
L = 512          # LQ == LK
D = 512          # DQ == DK == DV
H = 128
P = 128          # SBUF partitions
NDB = D // P     # 4 d-blocks
NQB = L // P     # 4 query blocks

# Optimized sine fit of tanh (offline, Gaussian weight sigma=1.665 + floor,
# range +-11; see module docstring).  The frequencies are stored as
# omega/2pi values EXACT in bf16, so the PE diag matmul that scales the
# projections introduces no frequency error; beta is refit against them.
M_FREQ = 5
WP = np.array([0.041259765625, 0.1259765625, 0.2265625,
               0.353515625, 0.5078125])           # omega_m / 2pi, bf16-exact
BETA = np.array([1.2385136240851755, 0.35480276158259183,
                 0.15381870648783913, 0.05379897839170714,
                 0.01455732383174043])

TWO_PI = float(2 * np.pi)
RND_C = float(1.5 * 2 ** 23)   # fp32 magic rounding constant
EXP_BIAS = -4.0                # constant softmax shift (cancels in normalize)

_CACHED_NC = None


def _build_nc():
    nc = bacc.Bacc("TRN2", target_bir_lowering=False, debug=False)

    # All inputs arrive pre-tiled on the host so each SBUF partition's data is
    # one contiguous DRAM line.
    qT = nc.dram_tensor("qT", [P, NDB, L], BF16, kind="ExternalInput")
    kT = nc.dram_tensor("kT", [P, NDB, L], BF16, kind="ExternalInput")
    val = nc.dram_tensor("val", [P, NQB, D], BF16, kind="ExternalInput")
    w1T = nc.dram_tensor("w1T", [P, NDB, H], BF16, kind="ExternalInput")
    w2T = nc.dram_tensor("w2T", [P, NDB, H], BF16, kind="ExternalInput")
    # vbeta[h, m] = v[h]*beta[m]; vbeta2 = -2*vbeta
    vbeta = nc.dram_tensor("vbeta", [H, M_FREQ], F32, kind="ExternalInput")
    vbeta2 = nc.dram_tensor("vbeta2", [H, M_FREQ], F32, kind="ExternalInput")
    # dgw[:, m, :] = diag(WP[m]) for the PE argument-scaling matmuls
    dgw = nc.dram_tensor("dgw", [P, M_FREQ, P], BF16, kind="ExternalInput")
    attn = nc.dram_tensor("attn", [L, L], BF16, kind="ExternalOutput")
    ctxo = nc.dram_tensor("ctx", [L, L], BF16, kind="ExternalOutput")

    with tile.TileContext(nc) as tc:
        with (
            tc.tile_pool(name="const", bufs=1) as const,
            tc.tile_pool(name="tu", bufs=2) as tu_pool,
            tc.tile_pool(name="feat", bufs=3) as feat_pool,
            tc.tile_pool(name="p", bufs=4) as p_pool,
            tc.tile_pool(name="pt", bufs=8) as pt_pool,
            tc.tile_pool(name="outs", bufs=4) as out_pool,
            tc.tile_pool(name="stat", bufs=8) as stat_pool,
            tc.tile_pool(name="score_ps", bufs=4, space="PSUM") as score_ps_pool,
            tc.tile_pool(name="rc_ps", bufs=2, space="PSUM") as rc_ps_pool,
        ):
            # ---------------- prologue ----------------
            ident = const.tile([P, P], BF16)
            make_identity(nc, ident[:])
            neg4 = const.tile([P, 1], F32)
            nc.vector.memset(neg4[:], EXP_BIAS)
            rndc = const.tile([P, 1], F32)
            nc.vector.memset(rndc[:], RND_C)

            # PE pre-warm on the locally-built identity: starts the busy
            # streak immediately, without waiting for any input DMA, and
            # without delaying the projections behind a long warm queue.
            warm_ps = rc_ps_pool.tile([H, 2 * L], F32, tag="rc", name="warm_ps")
            for _ in range(8):
                nc.tensor.matmul(warm_ps[:, 0:P], ident[:], ident[:])

            w1T_sb = const.tile([P, NDB, H], BF16)
            w2T_sb = const.tile([P, NDB, H], BF16)
            vbeta_sb = const.tile([H, M_FREQ], F32)
            vbeta2_sb = const.tile([H, M_FREQ], F32)
            dgw_sb = const.tile([P, M_FREQ, P], BF16)
            nc.sync.dma_start(out=w1T_sb[:], in_=w1T[:])
            nc.scalar.dma_start(out=w2T_sb[:], in_=w2T[:])

            qT_sb = const.tile([P, NDB, L], BF16)
            kT_sb = const.tile([P, NDB, L], BF16)
            for db in range(NDB):
                nc.sync.dma_start(out=qT_sb[:, db, :], in_=qT[:, db, :])
                nc.scalar.dma_start(out=kT_sb[:, db, :], in_=kT[:, db, :])

            # Small tensors not needed until the m-pipeline: after the
            # projection-critical loads.
            nc.scalar.dma_start(out=vbeta_sb[:], in_=vbeta[:, :])
            nc.scalar.dma_start(out=vbeta2_sb[:], in_=vbeta2[:, :])
            nc.scalar.dma_start(out=dgw_sb[:], in_=dgw[:])

            # value is needed only by the tail context matmuls; load it after
            # the projection inputs, split across both HWDGE queues.
            val_sb = const.tile([P, NQB, D], BF16)
            nc.sync.dma_start(out=val_sb[:, : NQB // 2, :], in_=val[:, : NQB // 2, :])
            nc.scalar.dma_start(out=val_sb[:, NQB // 2 :, :], in_=val[:, NQB // 2 :, :])

            # ---------------- projections: qkp = [qpT | kpT] ----------------
            qkp = const.tile([H, 2 * L], F32)
            ps_q = score_ps_pool.tile([H, L], F32, tag="score", name="ps_q")
            ps_k = score_ps_pool.tile([H, L], F32, tag="score", name="ps_k")
            for db in range(NDB):
                nc.tensor.matmul(
                    ps_q[:], w1T_sb[:, db, :], qT_sb[:, db, :],
                    start=(db == 0), stop=(db == NDB - 1),
                )
                nc.tensor.matmul(
                    ps_k[:], w2T_sb[:, db, :], kT_sb[:, db, :],
                    start=(db == 0), stop=(db == NDB - 1),
                )
            # PSUM->SBUF copies stay on DVE: a Copy on the ACT engine forces
            # an activation-table reload right before the first Sin.
            nc.vector.tensor_copy(qkp[:, 0:L], ps_q[:])
            nc.vector.tensor_copy(qkp[:, L : 2 * L], ps_k[:])
            # hi/lo bf16 split of qkp: diag(wp)*(hi+lo) on the PE reproduces
            # wp*qkp to ~2^-17 relative, at bf16 matmul speed.
            qkp_hi = const.tile([H, 2 * L], BF16)
            qkp_lo = const.tile([H, 2 * L], BF16)
            nc.vector.tensor_copy(qkp_hi[:], qkp[:])
            nc.vector.tensor_tensor(
                qkp_lo[:], qkp[:], qkp_hi[:], mybir.AluOpType.subtract
            )

            # ---------------- m-pipeline ----------------
            score_ps = [
                score_ps_pool.tile([P, L], F32, name=f"score_ps{qb}", tag="score")
                for qb in range(NQB)
            ]

            def emit_scores(m, sin_t, cv_t):
                # score += sin_q x (vb cos_k)  +  (vb cos_q) x sin_k
                for qb in range(NQB):
                    nc.tensor.matmul(
                        score_ps[qb][:],
                        sin_t[:, qb * P : (qb + 1) * P],
                        cv_t[:, L : 2 * L],
                        start=(m == 0), stop=False,
                    )
                    nc.tensor.matmul(
                        score_ps[qb][:],
                        cv_t[:, qb * P : (qb + 1) * P],
                        sin_t[:, L : 2 * L],
                        start=False, stop=(m == M_FREQ - 1),
                    )

            prev = None
            for m in range(M_FREQ):
                wp = float(WP[m])
                if m == 0:
                    # |omega_0 * x| < pi: no range reduction needed.
                    rc_src = qkp[:]
                    s_sin, s_h = TWO_PI * wp, float(np.pi) * wp
                else:
                    # k = round(wp*qkp) via fp32 magic rounding on DVE; the
                    # PE then accumulates rc = wp*(hi+lo) - k in PSUM.
                    ub_t = tu_pool.tile([H, 2 * L], F32, name="ub_t", tag="ub")
                    kneg_t = tu_pool.tile([H, 2 * L], BF16, name="kneg_t",
                                          tag="kneg")
                    nc.vector.tensor_scalar(
                        ub_t[:], qkp[:], wp, RND_C,
                        mybir.AluOpType.mult, mybir.AluOpType.add,
                    )
                    nc.vector.tensor_scalar(
                        kneg_t[:], ub_t[:], -1.0, RND_C,
                        mybir.AluOpType.mult, mybir.AluOpType.add,
                    )
                    rc_ps = rc_ps_pool.tile([H, 2 * L], F32, name="rc_ps",
                                            tag="rc")
                    for half in range(2):
                        sl = slice(half * L, (half + 1) * L)
                        nc.tensor.matmul(
                            rc_ps[:, sl], dgw_sb[:, m, :], qkp_hi[:, sl],
                            start=True, stop=False,
                        )
                        nc.tensor.matmul(
                            rc_ps[:, sl], dgw_sb[:, m, :], qkp_lo[:, sl],
                            start=False, stop=False,
                        )
                        nc.tensor.matmul(
                            rc_ps[:, sl], ident[:], kneg_t[:, sl],
                            start=False, stop=True,
                        )
                    rc_src = rc_ps[:]
                    s_sin, s_h = TWO_PI, float(np.pi)
                sin_t = feat_pool.tile([H, 2 * L], BF16, name="sin_t", tag="sin")
                h_t = feat_pool.tile([H, 2 * L], FP16, name="h_t", tag="h")
                nc.scalar.activation(
                    sin_t[:], rc_src, mybir.ActivationFunctionType.Sin,
                    scale=s_sin,
                )
                nc.scalar.activation(
                    h_t[:], rc_src, mybir.ActivationFunctionType.Sin,
                    scale=s_h,
                )
                h2_t = feat_pool.tile([H, 2 * L], FP16, name="h2_t", tag="h2")
                nc.vector.tensor_tensor(
                    h2_t[:], h_t[:], h_t[:], mybir.AluOpType.mult
                )
                cv_t = feat_pool.tile([H, 2 * L], BF16, name="cv_t", tag="cv")
                nc.vector.tensor_scalar(
                    cv_t[:], h2_t[:],
                    vbeta2_sb[:, m : m + 1], vbeta_sb[:, m : m + 1],
                    mybir.AluOpType.mult, mybir.AluOpType.add,
                )
                if prev is not None:
                    emit_scores(*prev)
                prev = (m, sin_t, cv_t)
            emit_scores(*prev)

            # ---------------- softmax + context per query block -------------
            for qb in range(NQB):
                p_t = p_pool.tile([P, L], BF16)
                sums = stat_pool.tile([P, 1], F32)
                nc.scalar.activation(
                    p_t[:],
                    score_ps[qb][:],
                    mybir.ActivationFunctionType.Exp,
                    bias=neg4[:],
                    accum_out=sums[:],
                )
                inv = stat_pool.tile([P, 1], F32)
                nc.vector.reciprocal(inv[:], sums[:])

                attn_t = out_pool.tile([P, L], BF16)
                nc.vector.tensor_scalar_mul(attn_t[:], p_t[:], inv[:])
                nc.sync.dma_start(
                    out=attn[qb * P : (qb + 1) * P, :], in_=attn_t[:]
                )

                # context: ctx[qb] = (p @ value) * inv
                pT_sbs = []
                for kb in range(NQB):
                    # Transposes rotate through the score banks, which free
                    # up as each block's exp consumes them.
                    tp = score_ps_pool.tile([P, P], BF16, name="tp", tag="score")
                    nc.tensor.transpose(
                        tp[:], p_t[:, kb * P : (kb + 1) * P], ident[:]
                    )
                    pT_sb = pt_pool.tile([P, P], BF16, name="pT_sb", tag="pt")
                    nc.vector.tensor_copy(pT_sb[:], tp[:])
                    pT_sbs.append(pT_sb)
                # ctx accumulators rotate through the (now dead) rc banks.
                ctx_ps = rc_ps_pool.tile([P, D], F32, tag="rc", name="ctx_ps")
                for kb in range(NQB):
                    nc.tensor.matmul(
                        ctx_ps[:],
                        pT_sbs[kb][:],
                        val_sb[:, kb, :],
                        start=(kb == 0),
                        stop=(kb == NQB - 1),
                    )
                ctx_t = out_pool.tile([P, D], BF16)
                nc.vector.tensor_scalar_mul(ctx_t[:], ctx_ps[:], inv[:])
                nc.scalar.dma_start(
                    out=ctxo[qb * P : (qb + 1) * P, :], in_=ctx_t[:]
                )

    nc.compile()
    return nc


def _get_nc():
    global _CACHED_NC
    if _CACHED_NC is None:
        _CACHED_NC = _build_nc()
    return _CACHED_NC


def _in_maps(query, key, value, w1, w2, v):
    import ml_dtypes as _md

    f = np.float32
    bf = _md.bfloat16

    def tile_rows(arr):
        # [R, C] with R = NB*P  ->  [P, NB, C]: partition-major, so each
        # SBUF partition's data is one contiguous DRAM line.
        r, c = arr.shape
        nb = r // P
        return np.ascontiguousarray(arr.reshape(nb, P, c).transpose(1, 0, 2))

    w1T = tile_rows(np.asarray(w1, dtype=f).T.astype(bf))
    w2T = tile_rows(np.asarray(w2, dtype=f).T.astype(bf))
    vb = (np.asarray(v, dtype=np.float64)[0][:, None] * BETA[None, :]).astype(f)
    vb2 = (-2.0 * vb).astype(f)
    dgw = np.zeros((P, M_FREQ, P), dtype=bf)
    for m in range(M_FREQ):
        np.fill_diagonal(dgw[:, m, :], bf(WP[m]))
    maps = []
    for b in range(B):
        maps.append(
            {
                "qT": tile_rows(np.asarray(query[b], dtype=f).T.astype(bf)),
                "kT": tile_rows(np.asarray(key[b], dtype=f).T.astype(bf)),
                "val": tile_rows(np.asarray(value[b], dtype=f).astype(bf)),
                "w1T": w1T,
                "w2T": w2T,
                "vbeta": vb,
                "vbeta2": vb2,
                "dgw": dgw,
            }
        )
    return maps


def run(query, key, value, w1, w2, v, trace=False, **spmd_kwargs):
    nc = _get_nc()
    res = run_bass_kernel_spmd(
        nc,
        _in_maps(query, key, value, w1, w2, v),
        list(range(B)),
        trace=trace,
        **spmd_kwargs,
    )
    attn = np.stack(
        [res.results[b]["attn"].astype(np.float32) for b in range(B)]
    )
    ctx = np.stack(
        [res.results[b]["ctx"].astype(np.float32) for b in range(B)]
    )
    return (attn, ctx), res


def kernel(query, key, value, w1, w2, v):
    (attn, ctx), _ = run(query, key, value, w1, w2, v, trace=False)
    return (attn, ctx)


# revision 41
# speedup vs baseline: 1.0636x; 1.0297x over previous
"""Bahdanau attention kernel for Trainium2 (Bass/Tile), 8 NeuronCores.

Problem (per batch element b):
    q_proj = query[b] @ w1.T          # (LQ, H)
    k_proj = key[b]   @ w2.T          # (LK, H)
    score[q, k] = sum_h v[h] * tanh(q_proj[q, h] + k_proj[k, h])
    attn = softmax(score, axis=-1)    # output 1
    ctx  = attn @ value[b]            # output 2

Sharding: data-parallel over batch B=8 across the 8 cores (no collectives).

Algorithm: instead of materializing the (LQ, LK, H) tanh tensor (33.5M
elements/core, ACT-bound at ~200us), expand tanh in a sine series fitted
under the Gaussian input distribution (weighted nonlinear LSQ offline):

    tanh(x) ~= sum_m beta_m sin(omega_m x)
    sin(w(a+b)) = sin(wa)cos(wb) + cos(wa)sin(wb)

so the score becomes a rank-(2*M*H) matmul between per-side feature maps
of the SMALL (H, L) projections.  Weighted-RMS fit error 1.2e-3 (M=6),
below the bf16 feature-quantization floor.

The HW Sin activation has no range reduction (valid only |arg| <~ pi), so
arguments are reduced explicitly per frequency with exact fp32 arithmetic:
    t  = qkp * (omega/2pi)                 (Pool)
    u  = (t + 1.5*2^23) - 1.5*2^23         (Pool; IEEE round-to-nearest)
    rc = t - u  in [-0.5, 0.5]             (DVE)
    sin = Sin(2pi * rc)                    (ACT, bf16)
    h   = Sin(pi * rc)                     (ACT, fp16)
    h2  = h * h                            (DVE, fp16)
    cos_scaled = vbeta - 2*vbeta*h2        (DVE; = v*beta*cos(w x), bf16)
The v_h*beta_m weights ride on both cos maps (one per side), so sin maps
stay unscaled and each m needs only 2 ACT + 3 DVE/Pool-pairable ALU ops.

Softmax uses a constant bias (scores are bounded, |score| < 4, and any
constant cancels in softmax), so there is no row-max reduction; exp writes
bf16 p directly (bf16 transposes, no casts) with an fp32 accumulator for
the row sums.  Outputs are written bf16 and upcast on the host.
"""

import numpy as np

import concourse.bass as bass
import concourse.mybir as mybir
import concourse.tile as tile
from concourse import bacc
from concourse.bass_utils import run_bass_kernel_spmd
from concourse.masks import make_identity

F32 = mybir.dt.float32
BF16 = mybir.dt.bfloat16
FP16 = mybir.dt.float16

B = 8
L = 512          # LQ == LK
D = 512          # DQ == DK == DV
H = 128
P = 128          # SBUF partitions
NDB = D // P     # 4 d-blocks
NQB = L // P     # 4 query blocks

# Optimized sine fit of tanh (offline, Gaussian weight sigma=1.665 + floor,
# range +-11; see module docstring).  The frequencies are stored as
# omega/2pi values EXACT in bf16, so the PE diag matmul that scales the
# projections introduces no frequency error; beta is refit against them.
M_FREQ = 5
WP = np.array([0.041259765625, 0.1259765625, 0.2265625,
               0.353515625, 0.5078125])           # omega_m / 2pi, bf16-exact
BETA = np.array([1.2385136240851755, 0.35480276158259183,
                 0.15381870648783913, 0.05379897839170714,
                 0.01455732383174043])

TWO_PI = float(2 * np.pi)
RND_C = float(1.5 * 2 ** 23)   # fp32 magic rounding constant
EXP_BIAS = -4.0                # constant softmax shift (cancels in normalize)

_CACHED_NC = None


def _build_nc():
    nc = bacc.Bacc("TRN2", target_bir_lowering=False, debug=False)

    # All inputs arrive pre-tiled on the host so each SBUF partition's data is
    # one contiguous DRAM line.
    qT = nc.dram_tensor("qT", [P, NDB, L], BF16, kind="ExternalInput")
    kT = nc.dram_tensor("kT", [P, NDB, L], BF16, kind="ExternalInput")
    val = nc.dram_tensor("val", [P, NQB, D], BF16, kind="ExternalInput")
    w1T = nc.dram_tensor("w1T", [P, NDB, H], BF16, kind="ExternalInput")
    w2T = nc.dram_tensor("w2T", [P, NDB, H], BF16, kind="ExternalInput")
    # vbeta[h, m] = v[h]*beta[m]; vbeta2 = -2*vbeta
    vbeta = nc.dram_tensor("vbeta", [H, M_FREQ], F32, kind="ExternalInput")
    vbeta2 = nc.dram_tensor("vbeta2", [H, M_FREQ], F32, kind="ExternalInput")
    # dgw[:, m, :] = diag(WP[m]) for the PE argument-scaling matmuls
    dgw = nc.dram_tensor("dgw", [P, M_FREQ, P], BF16, kind="ExternalInput")
    attn = nc.dram_tensor("attn", [L, L], BF16, kind="ExternalOutput")
    ctxo = nc.dram_tensor("ctx", [L, L], BF16, kind="ExternalOutput")

    with tile.TileContext(nc) as tc:
        with (
            tc.tile_pool(name="const", bufs=1) as const,
            tc.tile_pool(name="tu", bufs=2) as tu_pool,
            tc.tile_pool(name="feat", bufs=3) as feat_pool,
            tc.tile_pool(name="p", bufs=4) as p_pool,
            tc.tile_pool(name="pt", bufs=8) as pt_pool,
            tc.tile_pool(name="outs", bufs=4) as out_pool,
            tc.tile_pool(name="stat", bufs=8) as stat_pool,
            tc.tile_pool(name="score_ps", bufs=4, space="PSUM") as score_ps_pool,
            tc.tile_pool(name="rc_ps", bufs=2, space="PSUM") as rc_ps_pool,
        ):
            # ---------------- prologue ----------------
            ident = const.tile([P, P], BF16)
            make_identity(nc, ident[:])
            neg4 = const.tile([P, 1], F32)
            nc.vector.memset(neg4[:], EXP_BIAS)
            rndc = const.tile([P, 1], F32)
            nc.vector.memset(rndc[:], RND_C)

            # PE pre-warm on the locally-built identity: starts the busy
            # streak immediately, without waiting for any input DMA, and
            # without delaying the projections behind a long warm queue.
            warm_ps = rc_ps_pool.tile([H, 2 * L], F32, tag="rc", name="warm_ps")
            for _ in range(8):
                nc.tensor.matmul(warm_ps[:, 0:P], ident[:], ident[:])

            w1T_sb = const.tile([P, NDB, H], BF16)
            w2T_sb = const.tile([P, NDB, H], BF16)
            vbeta_sb = const.tile([H, M_FREQ], F32)
            vbeta2_sb = const.tile([H, M_FREQ], F32)
            dgw_sb = const.tile([P, M_FREQ, P], BF16)
            nc.sync.dma_start(out=w1T_sb[:], in_=w1T[:])
            nc.scalar.dma_start(out=w2T_sb[:], in_=w2T[:])

            qT_sb = const.tile([P, NDB, L], BF16)
            kT_sb = const.tile([P, NDB, L], BF16)
            for db in range(NDB):
                nc.sync.dma_start(out=qT_sb[:, db, :], in_=qT[:, db, :])
                nc.scalar.dma_start(out=kT_sb[:, db, :], in_=kT[:, db, :])

            # Small tensors not needed until the m-pipeline: after the
            # projection-critical loads.
            nc.scalar.dma_start(out=vbeta_sb[:], in_=vbeta[:, :])
            nc.scalar.dma_start(out=vbeta2_sb[:], in_=vbeta2[:, :])
            nc.scalar.dma_start(out=dgw_sb[:], in_=dgw[:])

            # value is needed only by the tail context matmuls; load it after
            # the projection inputs, split across both HWDGE queues.
            val_sb = const.tile([P, NQB, D], BF16)
            nc.sync.dma_start(out=val_sb[:, : NQB // 2, :], in_=val[:, : NQB // 2, :])
            nc.scalar.dma_start(out=val_sb[:, NQB // 2 :, :], in_=val[:, NQB // 2 :, :])

            # ---------------- projections: qkp = [qpT | kpT] ----------------
            qkp = const.tile([H, 2 * L], F32)
            ps_q = score_ps_pool.tile([H, L], F32, tag="score", name="ps_q")
            ps_k = score_ps_pool.tile([H, L], F32, tag="score", name="ps_k")
            for db in range(NDB):
                nc.tensor.matmul(
                    ps_q[:], w1T_sb[:, db, :], qT_sb[:, db, :],
                    start=(db == 0), stop=(db == NDB - 1),
                )
                nc.tensor.matmul(
                    ps_k[:], w2T_sb[:, db, :], kT_sb[:, db, :],
                    start=(db == 0), stop=(db == NDB - 1),
                )
            # PSUM->SBUF copies stay on DVE: a Copy on the ACT engine forces
            # an activation-table reload right before the first Sin.
            nc.vector.tensor_copy(qkp[:, 0:L], ps_q[:])
            nc.vector.tensor_copy(qkp[:, L : 2 * L], ps_k[:])
            # hi/lo bf16 split of qkp: diag(wp)*(hi+lo) on the PE reproduces
            # wp*qkp to ~2^-17 relative, at bf16 matmul speed.
            qkp_hi = const.tile([H, 2 * L], BF16)
            qkp_lo = const.tile([H, 2 * L], BF16)
            nc.vector.tensor_copy(qkp_hi[:], qkp[:])
            nc.vector.tensor_tensor(
                qkp_lo[:], qkp[:], qkp_hi[:], mybir.AluOpType.subtract
            )

            # ---------------- m-pipeline ----------------
            score_ps = [
                score_ps_pool.tile([P, L], F32, name=f"score_ps{qb}", tag="score")
                for qb in range(NQB)
            ]

            def emit_tail(m, sin_t, h_t):
                h2_t = feat_pool.tile([H, 2 * L], FP16, name="h2_t", tag="h2")
                nc.vector.tensor_tensor(
                    h2_t[:], h_t[:], h_t[:], mybir.AluOpType.mult
                )
                cv_t = feat_pool.tile([H, 2 * L], BF16, name="cv_t", tag="cv")
                nc.vector.tensor_scalar(
                    cv_t[:], h2_t[:],
                    vbeta2_sb[:, m : m + 1], vbeta_sb[:, m : m + 1],
                    mybir.AluOpType.mult, mybir.AluOpType.add,
                )
                emit_scores(m, sin_t, cv_t)

            def emit_scores(m, sin_t, cv_t):
                # score += sin_q x (vb cos_k)  +  (vb cos_q) x sin_k
                for qb in range(NQB):
                    nc.tensor.matmul(
                        score_ps[qb][:],
                        sin_t[:, qb * P : (qb + 1) * P],
                        cv_t[:, L : 2 * L],
                        start=(m == 0), stop=False,
                    )
                    nc.tensor.matmul(
                        score_ps[qb][:],
                        cv_t[:, qb * P : (qb + 1) * P],
                        sin_t[:, L : 2 * L],
                        start=False, stop=(m == M_FREQ - 1),
                    )

            prev = None
            for m in range(M_FREQ):
                wp = float(WP[m])
                if m == 0:
                    # |omega_0 * x| < pi: no range reduction needed.
                    rc_src = qkp[:]
                    s_sin, s_h = TWO_PI * wp, float(np.pi) * wp
                else:
                    # k = round(wp*qkp) via fp32 magic rounding on DVE; the
                    # PE then accumulates rc = wp*(hi+lo) - k in PSUM.
                    ub_t = tu_pool.tile([H, 2 * L], F32, name="ub_t", tag="ub")
                    kneg_t = tu_pool.tile([H, 2 * L], BF16, name="kneg_t",
                                          tag="kneg")
                    nc.vector.tensor_scalar(
                        ub_t[:], qkp[:], wp, RND_C,
                        mybir.AluOpType.mult, mybir.AluOpType.add,
                    )
                    nc.vector.tensor_scalar(
                        kneg_t[:], ub_t[:], -1.0, RND_C,
                        mybir.AluOpType.mult, mybir.AluOpType.add,
                    )
                    rc_ps = rc_ps_pool.tile([H, 2 * L], F32, name="rc_ps",
                                            tag="rc")
                    for half in range(2):
                        sl = slice(half * L, (half + 1) * L)
                        nc.tensor.matmul(
                            rc_ps[:, sl], dgw_sb[:, m, :], qkp_hi[:, sl],
                            start=True, stop=False,
                        )
                        nc.tensor.matmul(
                            rc_ps[:, sl], dgw_sb[:, m, :], qkp_lo[:, sl],
                            start=False, stop=False,
                        )
                        nc.tensor.matmul(
                            rc_ps[:, sl], ident[:], kneg_t[:, sl],
                            start=False, stop=True,
                        )
                    rc_src = rc_ps[:]
                    s_sin, s_h = TWO_PI, float(np.pi)
                sin_t = feat_pool.tile([H, 2 * L], BF16, name="sin_t", tag="sin")
                h_t = feat_pool.tile([H, 2 * L], FP16, name="h_t", tag="h")
                nc.scalar.activation(
                    sin_t[:], rc_src, mybir.ActivationFunctionType.Sin,
                    scale=s_sin,
                )
                nc.scalar.activation(
                    h_t[:], rc_src, mybir.ActivationFunctionType.Sin,
                    scale=s_h,
                )
                # h2/cv/scores for the PREVIOUS m are emitted here, after the
                # next step's ub/kneg, so the DVE never stalls waiting for
                # ACT's h map — it works on step m+1's chain in the meantime.
                if prev is not None:
                    emit_tail(*prev)
                prev = (m, sin_t, h_t)
            emit_tail(*prev)

            # ---------------- softmax + context per query block -------------
            for qb in range(NQB):
                p_t = p_pool.tile([P, L], BF16)
                sums = stat_pool.tile([P, 1], F32)
                nc.scalar.activation(
                    p_t[:],
                    score_ps[qb][:],
                    mybir.ActivationFunctionType.Exp,
                    bias=neg4[:],
                    accum_out=sums[:],
                )
                inv = stat_pool.tile([P, 1], F32)
                nc.vector.reciprocal(inv[:], sums[:])

                attn_t = out_pool.tile([P, L], BF16)
                nc.vector.tensor_scalar_mul(attn_t[:], p_t[:], inv[:])
                nc.sync.dma_start(
                    out=attn[qb * P : (qb + 1) * P, :], in_=attn_t[:]
                )

                # context: ctx[qb] = (p @ value) * inv
                pT_sbs = []
                for kb in range(NQB):
                    # Transposes rotate through the score banks, which free
                    # up as each block's exp consumes them.
                    tp = score_ps_pool.tile([P, P], BF16, name="tp", tag="score")
                    nc.tensor.transpose(
                        tp[:], p_t[:, kb * P : (kb + 1) * P], ident[:]
                    )
                    pT_sb = pt_pool.tile([P, P], BF16, name="pT_sb", tag="pt")
                    nc.vector.tensor_copy(pT_sb[:], tp[:])
                    pT_sbs.append(pT_sb)
                # ctx accumulators rotate through the (now dead) rc banks.
                ctx_ps = rc_ps_pool.tile([P, D], F32, tag="rc", name="ctx_ps")
                for kb in range(NQB):
                    nc.tensor.matmul(
                        ctx_ps[:],
                        pT_sbs[kb][:],
                        val_sb[:, kb, :],
                        start=(kb == 0),
                        stop=(kb == NQB - 1),
                    )
                ctx_t = out_pool.tile([P, D], BF16)
                nc.vector.tensor_scalar_mul(ctx_t[:], ctx_ps[:], inv[:])
                nc.scalar.dma_start(
                    out=ctxo[qb * P : (qb + 1) * P, :], in_=ctx_t[:]
                )

    nc.compile()
    return nc


def _get_nc():
    global _CACHED_NC
    if _CACHED_NC is None:
        _CACHED_NC = _build_nc()
    return _CACHED_NC


def _in_maps(query, key, value, w1, w2, v):
    import ml_dtypes as _md

    f = np.float32
    bf = _md.bfloat16

    def tile_rows(arr):
        # [R, C] with R = NB*P  ->  [P, NB, C]: partition-major, so each
        # SBUF partition's data is one contiguous DRAM line.
        r, c = arr.shape
        nb = r // P
        return np.ascontiguousarray(arr.reshape(nb, P, c).transpose(1, 0, 2))

    w1T = tile_rows(np.asarray(w1, dtype=f).T.astype(bf))
    w2T = tile_rows(np.asarray(w2, dtype=f).T.astype(bf))
    vb = (np.asarray(v, dtype=np.float64)[0][:, None] * BETA[None, :]).astype(f)
    vb2 = (-2.0 * vb).astype(f)
    dgw = np.zeros((P, M_FREQ, P), dtype=bf)
    for m in range(M_FREQ):
        np.fill_diagonal(dgw[:, m, :], bf(WP[m]))
    maps = []
    for b in range(B):
        maps.append(
            {
                "qT": tile_rows(np.asarray(query[b], dtype=f).T.astype(bf)),
                "kT": tile_rows(np.asarray(key[b], dtype=f).T.astype(bf)),
                "val": tile_rows(np.asarray(value[b], dtype=f).astype(bf)),
                "w1T": w1T,
                "w2T": w2T,
                "vbeta": vb,
                "vbeta2": vb2,
                "dgw": dgw,
            }
        )
    return maps


def run(query, key, value, w1, w2, v, trace=False, **spmd_kwargs):
    nc = _get_nc()
    res = run_bass_kernel_spmd(
        nc,
        _in_maps(query, key, value, w1, w2, v),
        list(range(B)),
        trace=trace,
        **spmd_kwargs,
    )
    attn = np.stack(
        [res.results[b]["attn"].astype(np.float32) for b in range(B)]
    )
    ctx = np.stack(
        [res.results[b]["ctx"].astype(np.float32) for b in range(B)]
    )
    return (attn, ctx), res


def kernel(query, key, value, w1, w2, v):
    (attn, ctx), _ = run(query, key, value, w1, w2, v, trace=False)
    return (attn, ctx)
